# revision 1
# baseline (speedup 1.0000x reference)
"""Distributed causal multi-head attention kernel for one TRN2 chip (8 NeuronCores).

Problem shapes (hardcoded): x [2, 2048, 1024], 16 heads x 64 head-dim, f32 I/O.

Sharding strategy:
  - Heads sharded 2-per-core: each core computes Q/K/V projections and causal
    attention for its 2 heads over the full sequence (perfectly balanced).
  - Scores are computed TRANSPOSED (S^T [tk, tq]) so softmax needs no
    cross-partition reduction: P' = exp(S^T/8) elementwise (no max-subtract;
    values are small enough for f32/bf16), rowsums come from a ones-column
    appended to V in the P'V matmul (lhsT M=65), normalization multiplies by
    the DMA-partition-broadcast reciprocal rowsum.
  - Two per-batch AllGathers (explicitly dependency-ordered around the DRAM
    bounce buffers) convert head-sharding -> sequence-sharding of z^T; each
    core then computes the output projection for 256 rows of EACH batch at
    its own column offset (register-driven dynamic DMA), so the batch-0 half
    overlaps the batch-1 gather.
  - Host: converts to bf16, pre-transposes x, slices weights; scatters the
    8 cores' row slices back. All biases are applied exactly on device.
"""

import sys

import numpy as np
import ml_dtypes

sys.path.insert(0, "/opt/trn_rl_repo")

B, T, E, N, H = 2, 2048, 1024, 16, 64
NCORES = 8
HPC = N // NCORES          # 2 heads per core
HL = HPC * H               # 128: local head width
BT = B * T                 # 4096
ROWS = BT // NCORES        # 512: output rows per core
EC = E // 128              # 8 chunks of the embedding (contraction) dim
GC = (N * H) // 128        # 8 chunks of the flattened head dim
TQ = 512                   # query tile (free dim of S^T / Z matmuls)
NQ = T // TQ               # 4 query tiles per batch
NKC = T // 128             # 16 key chunks per batch

BF16 = ml_dtypes.bfloat16

_CACHE = {}


def _build():
    import concourse.bass as bass
    import concourse.mybir as mybir
    from concourse import bacc
    from concourse.tile import TileContext, add_dep_helper
    from concourse.masks import make_identity

    f32 = mybir.dt.float32
    bf16 = mybir.dt.bfloat16

    nc = bacc.Bacc("TRN2", num_devices=NCORES)

    xT_d = nc.dram_tensor("xT", [EC, B, 128, T], bf16, kind="ExternalInput")
    wq_d = nc.dram_tensor("wq", [EC, 128, HL], bf16, kind="ExternalInput")
    wk_d = nc.dram_tensor("wk", [EC, 128, HL], bf16, kind="ExternalInput")
    wv_d = nc.dram_tensor("wv", [EC, 128, HL], bf16, kind="ExternalInput")
    wo_d = nc.dram_tensor("wo", [GC, 128, E], bf16, kind="ExternalInput")
    bq_d = nc.dram_tensor("bq", [HL, 1], f32, kind="ExternalInput")
    bk_d = nc.dram_tensor("bk", [HL, 1], f32, kind="ExternalInput")
    bv_d = nc.dram_tensor("bv", [HL, 1], f32, kind="ExternalInput")
    bo_d = nc.dram_tensor("bo", [1, E], bf16, kind="ExternalInput")
    cm_d = nc.dram_tensor("cmask", [4, 128, TQ], bf16, kind="ExternalInput")
    out_d = nc.dram_tensor("out", [ROWS, E], f32, kind="ExternalOutput")
    sel_d = nc.dram_tensor("sel", [1, 2], mybir.dt.uint32, kind="ExternalInput")
    ag_in = [
        nc.dram_tensor(f"ag_in{b}", [HL, T], bf16, kind="Internal")
        for b in range(B)
    ]
    ag_out = [
        nc.dram_tensor(
            f"ag_out{b}", [NCORES, HL, T], bf16, kind="Internal",
            addr_space="Shared",
        )
        for b in range(B)
    ]

    with TileContext(nc) as tc:
        with (
            tc.tile_pool(name="singles", bufs=1) as singles,
            tc.tile_pool(name="ptiles", bufs=6) as ptiles,
            tc.tile_pool(name="ztiles", bufs=6) as ztiles,
            tc.tile_pool(name="rtiles", bufs=6) as rtiles,
            tc.tile_pool(name="otiles", bufs=4) as otiles,
            tc.tile_pool(name="dscratch", bufs=8, space="DRAM") as dscratch,
            tc.tile_pool(name="psum", bufs=4, space="PSUM") as psum,
            tc.tile_pool(name="psum2", bufs=2, space="PSUM") as psum2,
        ):
            # ---- resident inputs ----
            # per-chunk tiles + weights-before-x DMA order so the first
            # projection matmul can start as soon as chunk 0 lands
            xT = [
                [
                    singles.tile([128, T], bf16, name=f"xT{k}_{b}")
                    for b in range(B)
                ]
                for k in range(EC)
            ]
            wq = [singles.tile([128, HL], bf16, name=f"wq{k}") for k in range(EC)]
            wk = [singles.tile([128, HL], bf16, name=f"wk{k}") for k in range(EC)]
            wv = [singles.tile([128, HL], bf16, name=f"wv{k}") for k in range(EC)]
            bq = singles.tile([HL, 1], f32)
            bk = singles.tile([HL, 1], f32)
            bv = singles.tile([HL, 1], f32)
            nc.sync.dma_start(out=bq, in_=bq_d[:])
            nc.sync.dma_start(out=bk, in_=bk_d[:])
            nc.sync.dma_start(out=bv, in_=bv_d[:])
            for k in range(EC):
                nc.sync.dma_start(out=wq[k], in_=wq_d[k])
                nc.sync.dma_start(out=wk[k], in_=wk_d[k])
                nc.sync.dma_start(out=wv[k], in_=wv_d[k])
            for b in range(B):
                for k in range(EC):
                    nc.sync.dma_start(out=xT[k][b], in_=xT_d[k, b])
            cm = singles.tile([128, 4, TQ], bf16)
            for j in range(4):
                nc.sync.dma_start(out=cm[:, j, :], in_=cm_d[j])
            ident = singles.tile([128, 128], bf16)
            make_identity(nc, ident)
            ones1 = singles.tile([1, 128], bf16)
            nc.vector.memset(ones1, 1.0)

            # ---- Q^T / K^T / V^T projections: [128(2hxH), T] per batch ----
            qT = [singles.tile([128, T], bf16, name=f"qT{b}") for b in range(B)]
            kT = [singles.tile([128, T], bf16, name=f"kT{b}") for b in range(B)]
            vT = [singles.tile([128, T], bf16, name=f"vT{b}") for b in range(B)]
            vp = [
                singles.tile([128, NKC, HPC, H + 1], bf16, name=f"vp{b}")
                for b in range(B)
            ]
            for b in range(B):
                nc.vector.memset(vp[b][:, :, :, H : H + 1], 1.0)
            for b in range(B):
                for name, w, dst, bias in (
                    ("q", wq, qT[b], bq),
                    ("k", wk, kT[b], bk),
                    ("v", wv, vT[b], bv),
                ):
                    for qt in range(T // TQ):
                        ps = psum.tile([128, TQ], f32, tag="ps", name="ps_prj")
                        for k in range(EC):
                            nc.tensor.matmul(
                                ps,
                                lhsT=w[k],
                                rhs=xT[k][b][:, qt * TQ : (qt + 1) * TQ],
                                start=(k == 0),
                                stop=(k == EC - 1),
                            )
                        # psum -> sbuf cast with exact per-partition bias add
                        nc.vector.tensor_scalar_add(
                            dst[:, qt * TQ : (qt + 1) * TQ], ps, bias
                        )
                # V' = [V | ones] per key chunk: vp [128(tk), chunk, head, 65]
                for c in range(NKC):
                    pst = psum.tile([128, 128], bf16, tag="ps", name="ps_tr")
                    nc.tensor.transpose(
                        pst, in_=vT[b][:, c * 128 : (c + 1) * 128], identity=ident
                    )
                    for h in range(HPC):
                        nc.vector.tensor_copy(
                            out=vp[b][:, c, h, 0:H], in_=pst[:, h * H : (h + 1) * H]
                        )

            # O-projection weights: emitted here so they load during the
            # attention phase's idle DMA time (needed right after the AG)
            wo = singles.tile([128, GC, E], bf16)
            for g in range(GC):
                nc.sync.dma_start(out=wo[:, g, :], in_=wo_d[g])
            bo = singles.tile([1, E], bf16)
            nc.sync.dma_start(out=bo, in_=bo_d[:])

            # ---- attention per (batch, query-tile) ----
            zz_stores = [[] for _ in range(B)]
            for b in range(B):
                for q in range(NQ):
                    zps = [
                        psum.tile([128, TQ], f32, tag="ps", name=f"zps{h}")
                        for h in range(HPC)
                    ]
                    nkeep = 4 * q + 4  # causal: key chunks 0..4q+3
                    for c in range(nkeep):
                        # diagonal chunks (j>=0): columns < j*128 are fully
                        # masked -> clip them out of S/exp/mask/Z entirely
                        j = c - 4 * q
                        lo = j * 128 if j >= 0 else 0
                        # both heads' scores into ONE 2-bank psum tile so a
                        # single exp covers them (amortizes ACT op overhead);
                        # the S pair runs concurrently in disjoint row groups
                        sps = psum2.tile([128, 2 * TQ], f32, tag="ps2", name="sps")
                        kcols = slice(c * 128, (c + 1) * 128)
                        for h in range(HPC):
                            hp = slice(h * H, (h + 1) * H)
                            nc.tensor.matmul(
                                sps[:, h * TQ + lo : (h + 1) * TQ],
                                lhsT=kT[b][hp, kcols],
                                rhs=qT[b][hp, q * TQ + lo : (q + 1) * TQ],
                                start=True,
                                stop=True,
                                tile_position=(h * H, 0),
                            )
                        pp = ptiles.tile([128, 2 * TQ], bf16, tag="pp")
                        if j < 3:
                            nc.scalar.activation(
                                pp[:, lo : 2 * TQ],
                                sps[:, lo : 2 * TQ],
                                mybir.ActivationFunctionType.Exp,
                                scale=0.125,
                            )
                        else:  # j=3: two ops beat exp-ing the 384-col gap
                            for h in range(HPC):
                                nc.scalar.activation(
                                    pp[:, h * TQ + lo : (h + 1) * TQ],
                                    sps[:, h * TQ + lo : (h + 1) * TQ],
                                    mybir.ActivationFunctionType.Exp,
                                    scale=0.125,
                                )
                        if j >= 0:  # causal mask on the diagonal blocks
                            for h in range(HPC):
                                nc.vector.tensor_mul(
                                    pp[:, h * TQ + lo : h * TQ + lo + 128],
                                    pp[:, h * TQ + lo : h * TQ + lo + 128],
                                    cm[:, 0, 0:128],
                                )
                        for h in range(HPC):
                            nc.tensor.matmul(
                                zps[h][0 : H + 1, lo:],
                                lhsT=vp[b][:, c, h, :],
                                rhs=pp[:, h * TQ + lo : (h + 1) * TQ],
                                start=(c == 0),
                                stop=(c == nkeep - 1),
                            )
                    # normalize: z = z' * (1/rowsum), rowsum is row H of zps
                    s = b * NQ + q  # destination core / row-slice id
                    # rowsum reciprocals: reshape [1, 2*TQ] onto 128 partitions
                    # via DRAM so the DVE reciprocal runs wide (free size 8).
                    rr = rtiles.tile([128, 2 * TQ], f32, tag="rr")
                    for h in range(HPC):
                        nc.vector.tensor_copy(
                            out=rr[H : H + 1, h * TQ : (h + 1) * TQ],
                            in_=zps[h][H : H + 1, :],
                        )
                    rsd = dscratch.tile([1, 2 * TQ], f32, tag="rsd")
                    nc.sync.dma_start(out=rsd, in_=rr[H : H + 1, :])
                    rq = rtiles.tile([128, 2 * TQ // 128], f32, tag="rq")
                    nc.sync.dma_start(
                        out=rq,
                        in_=rsd[0:1, :].rearrange("o (p f) -> (o p) f", p=128),
                    )
                    rqr = rtiles.tile([128, 2 * TQ // 128], f32, tag="rqr")
                    nc.vector.reciprocal(out=rqr, in_=rq)
                    rsd2 = dscratch.tile([1, 2 * TQ], f32, tag="rsd2")
                    nc.sync.dma_start(
                        out=rsd2[0:1, :].rearrange("o (p f) -> (o p) f", p=128),
                        in_=rqr,
                    )
                    for h in range(HPC):
                        rbc = rtiles.tile([H, TQ], f32, tag="rbc")
                        nc.sync.dma_start(
                            out=rbc,
                            in_=rsd2[
                                0:1, h * TQ : (h + 1) * TQ
                            ].partition_broadcast(H),
                        )
                        zz = ztiles.tile([H, TQ], bf16, tag="zz")
                        nc.vector.tensor_mul(zz, zps[h][0:H, :], rbc)
                        zz_stores[b].append(
                            nc.sync.dma_start(
                                out=ag_in[b][
                                    h * H : (h + 1) * H,
                                    q * TQ : (q + 1) * TQ,
                                ],
                                in_=zz,
                            )
                        )


            # ---- AllGather z^T across cores (one per batch) ----
            # explicit ordering: collective after all of its batch's z
            # stores; zo loads after the collective (DRAM deps are not
            # reliably tracked by Tile across the collective)
            ccs = []
            for b in range(B):
                cc = nc.gpsimd.collective_compute(
                    "AllGather",
                    mybir.AluOpType.bypass,
                    replica_groups=[list(range(NCORES))],
                    ins=[ag_in[b][:]],
                    outs=[ag_out[b][:]],
                )
                for d in zz_stores[b]:
                    add_dep_helper(cc.ins, d.ins, reason="ag after z stores")
                ccs.append(cc)
            # delay batch-0's gather past batch-1's mid-attention so the
            # collective's SDMA traffic doesn't contend with the normalize
            # DMA chains during the bulk of attention
            add_dep_helper(
                ccs[0].ins, zz_stores[1][3].ins, reason="delay cc0"
            )

            # ---- output projection for this core's 512-row slice ----
            # per-core slice select: each core takes HROWS rows from EACH
            # batch at the same column offset, so the batch-0 half of the
            # output projection overlaps the batch-1 AllGather
            HROWS = ROWS // B
            l_reg = nc.sync.alloc_register("sel_l")
            nc.sync.reg_load(l_reg, sel_d[0:1, 1:2])
            l_val = nc.sync.snap(
                l_reg, donate=True, min_val=0, max_val=T - HROWS
            )
            zo = [
                [
                    singles.tile([128, HROWS], bf16, name=f"zo{b}_{g}")
                    for g in range(GC)
                ]
                for b in range(B)
            ]
            for b in range(B):
                for g in range(GC):
                    zd = nc.sync.dma_start(
                        out=zo[b][g],
                        in_=ag_out[b][g, :, bass.ds(l_val, HROWS)],
                    )
                    add_dep_helper(zd.ins, ccs[b].ins, reason="zo after ag")
            for r in range(ROWS // 128):
                b2, r2 = r // (HROWS // 128), r % (HROWS // 128)
                for eh in range(E // 512):
                    po = psum.tile([128, 512], f32, tag="ps")
                    for g in range(GC):
                        nc.tensor.matmul(
                            po,
                            lhsT=zo[b2][g][:, r2 * 128 : (r2 + 1) * 128],
                            rhs=wo[:, g, eh * 512 : (eh + 1) * 512],
                            start=(g == 0),
                            stop=False,
                        )
                    nc.tensor.matmul(
                        po,
                        lhsT=ones1,
                        rhs=bo[:, eh * 512 : (eh + 1) * 512],
                        start=False,
                        stop=True,
                    )
                    ob = otiles.tile([128, 512], f32, tag="ob")
                    nc.vector.tensor_copy(out=ob, in_=po)
                    nc.sync.dma_start(
                        out=out_d[r * 128 : (r + 1) * 128, eh * 512 : (eh + 1) * 512],
                        in_=ob,
                    )

    nc.compile()
    return nc


def _prep_inputs(x, W_Q, W_K, W_V, W_O, b_Q, b_K, b_V, b_O):
    xT = np.ascontiguousarray(
        x.reshape(BT, E)
        .T.reshape(EC, 128, B, T)
        .transpose(0, 2, 1, 3)
        .astype(BF16)
    )
    wo = np.ascontiguousarray(
        W_O.reshape(N * H, E).reshape(GC, 128, E).astype(BF16)
    )
    bo = np.ascontiguousarray(b_O.reshape(1, E).astype(BF16))
    # causal mask variants for the 4 diagonal 128-col blocks of a 512 tile
    cmask = np.zeros((4, 128, TQ), dtype=BF16)
    cols = np.arange(TQ)[None, :]
    rows = np.arange(128)[:, None]
    for j in range(4):
        cmask[j] = (cols >= j * 128 + rows).astype(BF16)

    in_maps = []
    for i in range(NCORES):
        hs = slice(HPC * i, HPC * (i + 1))
        m = {
            "xT": xT,
            "wo": wo,
            "bo": bo,
            "cmask": cmask,
            "wq": np.ascontiguousarray(
                W_Q[hs].transpose(1, 0, 2).reshape(EC, 128, HL).astype(BF16)
            ),
            "wk": np.ascontiguousarray(
                W_K[hs].transpose(1, 0, 2).reshape(EC, 128, HL).astype(BF16)
            ),
            "wv": np.ascontiguousarray(
                W_V[hs].transpose(1, 0, 2).reshape(EC, 128, HL).astype(BF16)
            ),
            "bq": np.ascontiguousarray(
                b_Q[hs].reshape(HL, 1).astype(np.float32)
            ),
            "bk": np.ascontiguousarray(
                b_K[hs].reshape(HL, 1).astype(np.float32)
            ),
            "bv": np.ascontiguousarray(
                b_V[hs].reshape(HL, 1).astype(np.float32)
            ),
            "sel": np.array([[0, i * (ROWS // B)]], dtype=np.uint32),
        }
        in_maps.append(m)
    return in_maps


def run(inputs, trace=False):
    from concourse import bass_utils

    if "nc" not in _CACHE:
        _CACHE["nc"] = _build()
    nc = _CACHE["nc"]
    in_maps = _prep_inputs(**inputs)
    res = bass_utils.run_bass_kernel_spmd(
        nc, in_maps, core_ids=list(range(NCORES)), trace=trace
    )
    hr = ROWS // B
    out = np.zeros((B, T, E), dtype=np.float32)
    for i, r in enumerate(res.results):
        for b in range(B):
            out[b, i * hr : (i + 1) * hr] = r["out"][b * hr : (b + 1) * hr]
    return out, res


def kernel(**inputs):
    out, _ = run(inputs, trace=False)
    return out



# revision 3
# speedup vs baseline: 1.0663x; 1.0663x over previous
"""Distributed causal multi-head attention kernel for one TRN2 chip (8 NeuronCores).

Problem shapes (hardcoded): x [2, 2048, 1024], 16 heads x 64 head-dim, f32 I/O.

Sharding strategy:
  - Heads sharded 2-per-core: each core computes Q/K/V projections and causal
    attention for its 2 heads over the full sequence (perfectly balanced).
  - Scores are computed TRANSPOSED (S^T [tk, tq]) so softmax needs no
    cross-partition reduction: P' = exp(S^T/8) elementwise (no max-subtract;
    values are small enough for f32/bf16), rowsums come from a ones-column
    appended to V in the P'V matmul (lhsT M=65), normalization multiplies by
    a K=1-matmul partition-broadcast of the rowsum reciprocal (all on-chip,
    no DRAM bounces).
  - Two per-batch AllToAlls convert head-sharding -> sequence-sharding of
    z^T: each core sends 256-query-column slices of its 2 heads directly to
    the core that owns those output rows (8x less traffic than an
    AllGather), so the batch-0 exchange overlaps batch-1's attention and the
    per-core output slice needs no dynamic offset.
  - Each core then computes the output projection for its fixed 256 rows of
    EACH batch.
  - Host: converts to bf16, pre-transposes x, packs/slices weights; scatters
    the 8 cores' row slices back. All biases are applied exactly on device.
"""

import sys

import numpy as np
import ml_dtypes

sys.path.insert(0, "/opt/trn_rl_repo")

B, T, E, N, H = 2, 2048, 1024, 16, 64
NCORES = 8
HPC = N // NCORES          # 2 heads per core
HL = HPC * H               # 128: local head width
BT = B * T                 # 4096
ROWS = BT // NCORES        # 512: output rows per core
EC = E // 128              # 8 chunks of the embedding (contraction) dim
GC = (N * H) // 128        # 8 chunks of the flattened head dim
TQ = 512                   # query tile (free dim of S^T / Z matmuls)
NQ = T // TQ               # 4 query tiles per batch
NKC = T // 128             # 16 key chunks per batch
HROWS = ROWS // B          # 256: output rows per core per batch

BF16 = ml_dtypes.bfloat16

_CACHE = {}


def _build():
    import concourse.bass as bass
    import concourse.mybir as mybir
    from concourse import bacc
    from concourse.tile import TileContext, add_dep_helper
    from concourse.masks import make_identity

    f32 = mybir.dt.float32
    bf16 = mybir.dt.bfloat16

    nc = bacc.Bacc("TRN2", num_devices=NCORES)

    xT_d = nc.dram_tensor("xT", [EC, B, 128, T], bf16, kind="ExternalInput")
    wqkv_d = nc.dram_tensor("wqkv", [128, 3 * EC * 128], bf16, kind="ExternalInput")
    wo_d = nc.dram_tensor("wo", [128, GC * E], bf16, kind="ExternalInput")
    bqkv_d = nc.dram_tensor("bqkv", [HL, 3], f32, kind="ExternalInput")
    bo_d = nc.dram_tensor("bo", [1, E], bf16, kind="ExternalInput")
    cm_d = nc.dram_tensor("cmask", [128, 128], bf16, kind="ExternalInput")
    out_d = nc.dram_tensor("out", [ROWS, E], f32, kind="ExternalOutput")
    # AllToAll buffers: [consumer core, 128 (2h x H), 256 (q cols)]
    aa_in = [
        nc.dram_tensor(f"aa_in{b}", [NCORES, HL, HROWS], bf16, kind="Internal")
        for b in range(B)
    ]
    aa_out = [
        nc.dram_tensor(f"aa_out{b}", [NCORES, HL, HROWS], bf16, kind="Internal")
        for b in range(B)
    ]

    with TileContext(nc) as tc:
        with (
            tc.tile_pool(name="singles", bufs=1) as singles,
            tc.tile_pool(name="ptiles", bufs=6) as ptiles,
            tc.tile_pool(name="ztiles", bufs=6) as ztiles,
            tc.tile_pool(name="rtiles", bufs=4) as rtiles,
            tc.tile_pool(name="otiles", bufs=4) as otiles,
            tc.tile_pool(name="psum", bufs=4, space="PSUM") as psum,
            tc.tile_pool(name="psum2", bufs=2, space="PSUM") as psum2,
        ):
            # ---- resident inputs ----
            # packed weights land first so the first projection matmul can
            # start as soon as x's chunk 0 arrives (per-chunk DMA deps)
            wqkv = singles.tile([128, 3, EC, 128], bf16)
            nc.sync.dma_start(out=wqkv, in_=wqkv_d[:].rearrange("p (t k f) -> p t k f", t=3, k=EC))
            bqkv = singles.tile([HL, 3], f32)
            nc.sync.dma_start(out=bqkv, in_=bqkv_d[:])
            cm = singles.tile([128, 128], bf16)
            nc.sync.dma_start(out=cm, in_=cm_d[:])
            xT = [
                [
                    singles.tile([128, T], bf16, name=f"xT{k}_{b}")
                    for b in range(B)
                ]
                for k in range(EC)
            ]
            for b in range(B):
                for k in range(EC):
                    nc.sync.dma_start(out=xT[k][b], in_=xT_d[k, b])
            ident = singles.tile([128, 128], bf16)
            make_identity(nc, ident)
            ones1 = singles.tile([1, 128], bf16)
            nc.vector.memset(ones1, 1.0)

            # ---- Q^T / K^T / V^T projections: [128(2hxH), T] per batch ----
            qT = [singles.tile([128, T], bf16, name=f"qT{b}") for b in range(B)]
            kT = [singles.tile([128, T], bf16, name=f"kT{b}") for b in range(B)]
            vp = [
                singles.tile([128, NKC, HPC, H + 1], bf16, name=f"vp{b}")
                for b in range(B)
            ]
            for b in range(B):
                nc.vector.memset(vp[b][:, :, :, H : H + 1], 1.0)
            vT = [singles.tile([128, T], bf16, name=f"vT{b}") for b in range(B)]
            for b in range(B):
                for t, dst in ((0, qT[b]), (1, kT[b]), (2, vT[b])):
                    for qt in range(T // TQ):
                        ps = psum.tile([128, TQ], f32, tag="ps", name="ps_prj")
                        for k in range(EC):
                            nc.tensor.matmul(
                                ps,
                                lhsT=wqkv[:, t, k, :],
                                rhs=xT[k][b][:, qt * TQ : (qt + 1) * TQ],
                                start=(k == 0),
                                stop=(k == EC - 1),
                            )
                        # psum -> sbuf cast with exact per-partition bias add
                        nc.vector.tensor_scalar_add(
                            dst[:, qt * TQ : (qt + 1) * TQ], ps, bqkv[:, t : t + 1]
                        )
                # V' = [V | ones] per key chunk: vp [128(tk), chunk, head, 65]
                for c in range(NKC):
                    pst = psum.tile([128, 128], bf16, tag="ps", name="ps_tr")
                    nc.tensor.transpose(
                        pst, in_=vT[b][:, c * 128 : (c + 1) * 128], identity=ident
                    )
                    for h in range(HPC):
                        nc.vector.tensor_copy(
                            out=vp[b][:, c, h, 0:H], in_=pst[:, h * H : (h + 1) * H]
                        )

            # O-projection weights: emitted here so they load during the
            # attention phase's idle DMA time (needed right after the AllToAll)
            wo = singles.tile([128, GC, E], bf16)
            nc.sync.dma_start(out=wo, in_=wo_d[:].rearrange("p (g e) -> p g e", g=GC))
            bo = singles.tile([1, E], bf16)
            nc.sync.dma_start(out=bo, in_=bo_d[:])

            # ---- attention per (batch, query-tile) ----
            zz_stores = [[] for _ in range(B)]
            for b in range(B):
                for q in range(NQ):
                    zps = [
                        psum.tile([128, TQ], f32, tag="ps", name=f"zps{h}")
                        for h in range(HPC)
                    ]
                    nkeep = 4 * q + 4  # causal: key chunks 0..4q+3
                    for c in range(nkeep):
                        # diagonal chunks (j>=0): columns < j*128 are fully
                        # masked -> clip them out of S/exp/mask/Z entirely
                        j = c - 4 * q
                        lo = j * 128 if j >= 0 else 0
                        # both heads' scores into ONE 2-bank psum tile so a
                        # single exp covers them (amortizes ACT op overhead);
                        # the S pair runs concurrently in disjoint row groups
                        sps = psum2.tile([128, 2 * TQ], f32, tag="ps2", name="sps")
                        kcols = slice(c * 128, (c + 1) * 128)
                        for h in range(HPC):
                            hp = slice(h * H, (h + 1) * H)
                            nc.tensor.matmul(
                                sps[:, h * TQ + lo : (h + 1) * TQ],
                                lhsT=kT[b][hp, kcols],
                                rhs=qT[b][hp, q * TQ + lo : (q + 1) * TQ],
                                start=True,
                                stop=True,
                                tile_position=(h * H, 0),
                            )
                        pp = ptiles.tile([128, 2 * TQ], bf16, tag="pp")
                        if j < 3:
                            nc.scalar.activation(
                                pp[:, lo : 2 * TQ],
                                sps[:, lo : 2 * TQ],
                                mybir.ActivationFunctionType.Exp,
                                scale=0.125,
                            )
                        else:  # j=3: two ops beat exp-ing the 384-col gap
                            for h in range(HPC):
                                nc.scalar.activation(
                                    pp[:, h * TQ + lo : (h + 1) * TQ],
                                    sps[:, h * TQ + lo : (h + 1) * TQ],
                                    mybir.ActivationFunctionType.Exp,
                                    scale=0.125,
                                )
                        if j >= 0:  # causal mask on the diagonal blocks
                            for h in range(HPC):
                                nc.vector.tensor_mul(
                                    pp[:, h * TQ + lo : h * TQ + lo + 128],
                                    pp[:, h * TQ + lo : h * TQ + lo + 128],
                                    cm,
                                )
                        for h in range(HPC):
                            nc.tensor.matmul(
                                zps[h][0 : H + 1, lo:],
                                lhsT=vp[b][:, c, h, :],
                                rhs=pp[:, h * TQ + lo : (h + 1) * TQ],
                                start=(c == 0),
                                stop=(c == nkeep - 1),
                            )
                    # normalize: z = z' * (1/rowsum); rowsum is row H of zps.
                    # Broadcast across partitions via a K=1 matmul of the
                    # rowsum row against a ones column (all on-chip).
                    rsb = rtiles.tile([1, 2 * TQ], bf16, tag="rsb")
                    for h in range(HPC):
                        nc.vector.tensor_copy(
                            out=rsb[0:1, h * TQ : (h + 1) * TQ],
                            in_=zps[h][H : H + 1, :],
                        )
                    bc = psum2.tile([H, 2 * TQ], f32, tag="ps2", name="bc")
                    for h in range(HPC):  # one matmul per PSUM bank (N<=512)
                        nc.tensor.matmul(
                            bc[:, h * TQ : (h + 1) * TQ],
                            lhsT=ones1[0:1, 0:H],
                            rhs=rsb[:, h * TQ : (h + 1) * TQ],
                            start=True,
                            stop=True,
                        )
                    rbc = rtiles.tile([H, 2 * TQ], f32, tag="rbc")
                    nc.vector.reciprocal(out=rbc, in_=bc)
                    for h in range(HPC):
                        zz = ztiles.tile([H, TQ], bf16, tag="zz")
                        nc.vector.tensor_mul(
                            zz, zps[h][0:H, :], rbc[:, h * TQ : (h + 1) * TQ]
                        )
                        # scatter the two 256-col halves to their consumer
                        # cores' AllToAll chunks
                        for s in range(2):
                            zz_stores[b].append(
                                nc.sync.dma_start(
                                    out=aa_in[b][
                                        2 * q + s, h * H : (h + 1) * H, :
                                    ],
                                    in_=zz[:, s * HROWS : (s + 1) * HROWS],
                                )
                            )

            # ---- AllToAll z^T across cores (one per batch) ----
            # explicit ordering: collective after all of its batch's z
            # stores; zo loads after the collective (DRAM deps are not
            # reliably tracked by Tile across the collective)
            ccs = []
            for b in range(B):
                cc = nc.gpsimd.collective_compute(
                    "AllToAll",
                    mybir.AluOpType.bypass,
                    replica_groups=[list(range(NCORES))],
                    ins=[aa_in[b][:]],
                    outs=[aa_out[b][:]],
                )
                for d in zz_stores[b]:
                    add_dep_helper(cc.ins, d.ins, reason="aa after z stores")
                ccs.append(cc)

            # ---- output projection for this core's fixed 256-row slices ----
            # (AllToAll already routed exactly this core's columns here, so
            # the batch-0 half overlaps the batch-1 exchange)
            zo = [
                singles.tile([128, GC, HROWS], bf16, name=f"zo{b}")
                for b in range(B)
            ]
            for b in range(B):
                zd = nc.sync.dma_start(
                    out=zo[b],
                    in_=aa_out[b][:].rearrange("g p f -> p g f"),
                )
                add_dep_helper(zd.ins, ccs[b].ins, reason="zo after aa")
            for r in range(ROWS // 128):
                b2, r2 = r // (HROWS // 128), r % (HROWS // 128)
                for eh in range(E // 512):
                    po = psum.tile([128, 512], f32, tag="ps")
                    for g in range(GC):
                        nc.tensor.matmul(
                            po,
                            lhsT=zo[b2][:, g, r2 * 128 : (r2 + 1) * 128],
                            rhs=wo[:, g, eh * 512 : (eh + 1) * 512],
                            start=(g == 0),
                            stop=False,
                        )
                    nc.tensor.matmul(
                        po,
                        lhsT=ones1,
                        rhs=bo[:, eh * 512 : (eh + 1) * 512],
                        start=False,
                        stop=True,
                    )
                    ob = otiles.tile([128, 512], f32, tag="ob")
                    nc.vector.tensor_copy(out=ob, in_=po)
                    nc.sync.dma_start(
                        out=out_d[r * 128 : (r + 1) * 128, eh * 512 : (eh + 1) * 512],
                        in_=ob,
                    )

    nc.compile()
    return nc


def _prep_inputs(x, W_Q, W_K, W_V, W_O, b_Q, b_K, b_V, b_O):
    xT = np.ascontiguousarray(
        x.reshape(BT, E)
        .T.reshape(EC, 128, B, T)
        .transpose(0, 2, 1, 3)
        .astype(BF16)
    )
    # wo: [128 (row within head group), GC * E], group-major columns
    wo = np.ascontiguousarray(
        W_O.reshape(GC, 128, E).transpose(1, 0, 2).reshape(128, GC * E).astype(BF16)
    )
    bo = np.ascontiguousarray(b_O.reshape(1, E).astype(BF16))
    # lower-triangle [128,128] causal mask block (cols >= rows allowed)
    cols = np.arange(128)[None, :]
    rows = np.arange(128)[:, None]
    cmask = np.ascontiguousarray((cols >= rows).astype(BF16))

    def packw(W, hs):
        # [2, E, H] -> [128 (E row within chunk), EC, HL] flattened
        return (
            W[hs].transpose(1, 0, 2).reshape(EC, 128, HL)
            .transpose(1, 0, 2).reshape(128, EC * HL).astype(BF16)
        )

    in_maps = []
    for i in range(NCORES):
        hs = slice(HPC * i, HPC * (i + 1))
        wqkv = np.concatenate(
            [packw(W_Q, hs), packw(W_K, hs), packw(W_V, hs)], axis=1
        )
        bqkv = np.stack(
            [
                b_Q[hs].reshape(HL),
                b_K[hs].reshape(HL),
                b_V[hs].reshape(HL),
            ],
            axis=1,
        ).astype(np.float32)
        m = {
            "xT": xT,
            "wqkv": np.ascontiguousarray(wqkv),
            "bqkv": np.ascontiguousarray(bqkv),
            "wo": wo,
            "bo": bo,
            "cmask": cmask,
        }
        in_maps.append(m)
    return in_maps


def run(inputs, trace=False):
    from concourse import bass_utils

    if "nc" not in _CACHE:
        _CACHE["nc"] = _build()
    nc = _CACHE["nc"]
    in_maps = _prep_inputs(**inputs)
    res = bass_utils.run_bass_kernel_spmd(
        nc, in_maps, core_ids=list(range(NCORES)), trace=trace
    )
    out = np.zeros((B, T, E), dtype=np.float32)
    for i, r in enumerate(res.results):
        for b in range(B):
            out[b, i * HROWS : (i + 1) * HROWS] = r["out"][
                b * HROWS : (b + 1) * HROWS
            ]
    return out, res


def kernel(**inputs):
    out, _ = run(inputs, trace=False)
    return out
